# revision 2
# baseline (speedup 1.0000x reference)
"""Multi-head attention (B=2, N=2048, E=1024, H=16) on 8 Trainium2 NeuronCores.

v2: all-bf16 data path + contiguous host-side layouts.

Sharding: data-parallel over batch (2) x tensor-parallel over head-groups (4
groups of 4 heads).  Core c handles batch c//4 and heads 4*(c%4)..4*(c%4)+3.

Host pre-shapes every tensor into the exact SBUF layout ([partition, ...]
contiguous per partition) and casts to bf16, so each DMA is a handful of
large contiguous descriptors.  The device kernel computes
  qT = Wq_s @ xT + bq_s        (feature-major, bf16 [256, 2048])
  kT = Wk_s @ xT + bk_s
  v  = x @ Wv_s.T + bv_s       (position-major, [2048, 256], bf16)
  eT[kpos, q] per head          (transposed energy, f32 psum, K=64 bf16 mm)
  s = exp(eT)  (bf16)           (no max-subtraction: |logits| < ~60 << 88)
  o  = s.T @ [v | 32]           (32-column yields 32*rowsum in psum row 64)
  oT normalized by 1/(32*rowsum)   (= softmax / sqrt(E) module quirk)
  out_partial = oT.T @ Wp[:, cols].T   (position-major [2048, 1024])
Host sums the 4 head-group partials per batch and adds bp.

All matmuls run in bf16 (fp32 PSUM accumulation); fp32r is ~2x slower on HW.
q-chunk projections and the output projection are emitted interleaved with
the attention inner loop so the PE fills the gaps while ScalarE (exp) paces.
"""

import numpy as np

B, N, E, H = 2, 2048, 1024, 16
D = E // H           # 64
NCORES = 8
HG = 4               # head groups
DH = E // HG         # 256 features per head-group
P = 128
NCH = N // 512       # 4 n-chunks of 512
ECH = E // P         # 8 contraction chunks
DCH = DH // P        # 2 feature chunks per shard
KT = N // P          # 16 key tiles
SCALE_COL = float(E ** 0.5)   # 32.0; row 64 of po = 32*rowsum

INTERLEAVE = True   # ride qproj/outproj units inside the attention kt loop
DEBUG_DUMP = False  # add qT/kT/v/oT dram outputs for HW debugging

_CACHE = {}


def _build_program(repeat=1):
    import concourse.bacc as bacc
    import concourse.tile as tile
    from concourse import mybir

    F32 = mybir.dt.float32
    BF16 = mybir.dt.bfloat16
    EXP = mybir.ActivationFunctionType.Exp

    nc = bacc.Bacc(None, target_bir_lowering=False, debug=False)

    xqt = nc.declare_dram_parameter("xqt", [P, ECH, N], BF16, isOutput=False)
    xkt = nc.declare_dram_parameter("xkt", [P, ECH, N], BF16, isOutput=False)
    xvt = nc.declare_dram_parameter("xvt", [P, ECH, N], BF16, isOutput=False)
    wqt = nc.declare_dram_parameter("wqt", [P, ECH, DH], BF16, isOutput=False)
    wkt = nc.declare_dram_parameter("wkt", [P, ECH, DH], BF16, isOutput=False)
    wvt = nc.declare_dram_parameter("wvt", [P, ECH, DH], BF16, isOutput=False)
    wpt = nc.declare_dram_parameter("wpt", [P, DCH, E], BF16, isOutput=False)
    bqp = nc.declare_dram_parameter("bq", [DCH, P, 1], F32, isOutput=False)
    bkp = nc.declare_dram_parameter("bk", [DCH, P, 1], F32, isOutput=False)
    bvp = nc.declare_dram_parameter("bv", [1, DH], BF16, isOutput=False)
    out = nc.declare_dram_parameter("out", [N, E], F32, isOutput=True)
    if DEBUG_DUMP:
        qTd = nc.declare_dram_parameter("qTd", [P, DCH, N], BF16, isOutput=True)
        kTd = nc.declare_dram_parameter("kTd", [P, DCH, N], BF16, isOutput=True)
        vd = nc.declare_dram_parameter("vd", [P, KT, HG, D + 1], BF16, isOutput=True)
        oTd = nc.declare_dram_parameter("oTd", [P, DCH, N], BF16, isOutput=True)

    with tile.TileContext(nc) as tc:
        with (
            tc.tile_pool(name="singles", bufs=1) as singles,
            tc.tile_pool(name="spool", bufs=4) as spool,
            tc.tile_pool(name="npool", bufs=2) as npool,
            tc.tile_pool(name="opool", bufs=2) as opool,
            tc.tile_pool(name="pproj", bufs=1, space="PSUM") as pproj,
            tc.tile_pool(name="peps", bufs=2, space="PSUM") as peps,
            tc.tile_pool(name="ppo", bufs=2, space="PSUM") as ppo,
            tc.tile_pool(name="pbc", bufs=1, space="PSUM") as pbc,
        ):
            # ---- persistent tiles ----
            wq_sb = singles.tile([P, ECH, DH], BF16)
            wk_sb = singles.tile([P, ECH, DH], BF16)
            wv_sb = singles.tile([P, ECH, DH], BF16)
            wp_sb = singles.tile([P, DCH, E], BF16)
            bq_sb = singles.tile([P, DCH], F32)
            bk_sb = singles.tile([P, DCH], F32)
            bv_sb = singles.tile([1, DH], BF16)
            ones1 = singles.tile([1, P], BF16)
            nc.vector.memset(ones1, 1.0)
            ones1_f = singles.tile([1, P], F32)
            nc.vector.memset(ones1_f, 1.0)

            xk_sb = singles.tile([P, ECH, N], BF16)
            xv_sb = singles.tile([P, ECH, N], BF16)
            xq_sb = singles.tile([P, ECH, N], BF16)

            qT_sb = singles.tile([P, DCH, N], BF16)
            kT_sb = singles.tile([P, DCH, N], BF16)
            oT_sb = singles.tile([P, DCH, N], BF16)
            v_sb = singles.tile([P, KT, HG, D + 1], BF16)
            nc.vector.memset(v_sb[:, :, :, D : D + 1], SCALE_COL)

            def load_inputs():
                nc.sync.dma_start(out=wk_sb, in_=wkt[:, :, :])
                nc.sync.dma_start(out=wv_sb, in_=wvt[:, :, :])
                nc.sync.dma_start(out=wq_sb, in_=wqt[:, :, :])
                nc.sync.dma_start(out=wp_sb, in_=wpt[:, :, :])
                for c in range(DCH):
                    nc.sync.dma_start(out=bq_sb[:, c : c + 1], in_=bqp[c])
                    nc.sync.dma_start(out=bk_sb[:, c : c + 1], in_=bkp[c])
                nc.sync.dma_start(out=bv_sb, in_=bvp[:, :])
                nc.sync.dma_start(out=xk_sb, in_=xkt[:, :, :])
                nc.sync.dma_start(out=xv_sb, in_=xvt[:, :, :])
                nc.sync.dma_start(out=xq_sb, in_=xqt[:, :, :])

            # ---- emit helpers ----
            def proj_unit(x_sb, w_sb, b_sb, dst, ni, dc):
                """One [128, 512] feature-major projection chunk."""
                ns = slice(ni * 512, (ni + 1) * 512)
                ps = pproj.tile([P, 512], F32, tag="proj", name=f"ps{ni}{dc}")
                for ec in range(ECH):
                    nc.tensor.matmul(
                        ps,
                        w_sb[:, ec, dc * P : (dc + 1) * P],
                        x_sb[:, ec, ns],
                        start=(ec == 0),
                        stop=(ec == ECH - 1),
                    )
                nc.vector.tensor_scalar_add(dst[:, dc, ns], ps, b_sb[:, dc : dc + 1])

            def v_unit(kt):
                """One [128, 256] position-major v tile (with bias)."""
                vps = pproj.tile([P, DH], F32, tag="proj", name=f"vps{kt}")
                nc.tensor.matmul(vps, ones1, bv_sb, start=True, stop=False)
                for ec in range(ECH):
                    nc.tensor.matmul(
                        vps,
                        xv_sb[:, ec, kt * P : (kt + 1) * P],
                        wv_sb[:, ec, :],
                        start=False,
                        stop=(ec == ECH - 1),
                    )
                nc.vector.tensor_copy(
                    v_sb[:, kt, :, 0:D],
                    vps.rearrange("p (h d) -> p h d", h=HG),
                )

            def outproj_units(qc):
                """Deferred output projection for q-chunk qc: 4 n-tile units."""

                def unit(nt):
                    n0 = qc * 512 + nt * P
                    osb = opool.tile([P, E], F32, tag="osb", name=f"osb{qc}{nt}")
                    for ecx in range(2):
                        ops = pproj.tile(
                            [P, 512], F32, tag="proj", name=f"ops{qc}{nt}{ecx}"
                        )
                        for dc in range(DCH):
                            nc.tensor.matmul(
                                ops,
                                oT_sb[:, dc, n0 : n0 + P],
                                wp_sb[:, dc, ecx * 512 : (ecx + 1) * 512],
                                start=(dc == 0),
                                stop=(dc == DCH - 1),
                            )
                        nc.vector.tensor_copy(
                            osb[:, ecx * 512 : (ecx + 1) * 512], ops
                        )
                    nc.sync.dma_start(out=out[n0 : n0 + P, :], in_=osb)

                return [lambda nt=nt: unit(nt) for nt in range(4)]

            def attn_pass(qc, pr, po, slots=None):
                """Full attention pass over all 16 k-tiles for (qc, pr)."""
                qs = slice(qc * 512, (qc + 1) * 512)
                for kt in range(KT):
                    ks = slice(kt * P, (kt + 1) * P)
                    eps = peps.tile([P, 1024], F32, tag="eps", name=f"e{qc}{pr}{kt}")
                    for hp in range(2):
                        rows = slice(hp * D, (hp + 1) * D)
                        nc.tensor.matmul(
                            eps[:, hp * 512 : (hp + 1) * 512],
                            kT_sb[rows, pr, ks],
                            qT_sb[rows, pr, qs],
                            start=True,
                            stop=True,
                        )
                    sT = spool.tile([P, 1024], BF16, tag="sT", name=f"s{qc}{pr}{kt}")
                    nc.scalar.activation(sT, eps, EXP)
                    for hp in range(2):
                        nc.tensor.matmul(
                            po[hp],
                            v_sb[:, kt, 2 * pr + hp, :],
                            sT[:, hp * 512 : (hp + 1) * 512],
                            start=(kt == 0),
                            stop=(kt == KT - 1),
                        )
                    if slots is not None and kt < len(slots):
                        for u in slots[kt]:
                            u()

            def normalize(qc, pr, po):
                qs = slice(qc * 512, (qc + 1) * 512)
                for hp in range(2):
                    rinv = npool.tile([1, 512], F32, tag="rinv")
                    nc.vector.reciprocal(rinv, po[hp][D : D + 1, :])
                    o_tmp = npool.tile([D, 512], F32, tag="otmp")
                    nc.vector.tensor_copy(o_tmp, po[hp][0:D, :])
                    bc = pbc.tile([D, 512], F32, tag="bc")
                    nc.tensor.matmul(
                        bc, ones1_f[:, 0:D], rinv, start=True, stop=True
                    )
                    nc.vector.tensor_mul(
                        oT_sb[hp * D : (hp + 1) * D, pr, qs], o_tmp, bc
                    )

            def new_po(qc, pr):
                return [
                    ppo.tile([D + 1, 512], F32, tag="po", name=f"po{qc}{pr}{hp}")
                    for hp in range(2)
                ]

            # ---- emission ----
            # k, v, q(chunk 0) projections up front; the Tile scheduler
            # overlaps the x DMAs with the earliest independent matmuls.
            # repeat>1 re-emits the whole body for device-time slope probes.
            for _rep in range(repeat):
              load_inputs()
              for ni in range(NCH):
                for dc in range(DCH):
                    proj_unit(xk_sb, wk_sb, bk_sb, kT_sb, ni, dc)
              for kt in range(KT):
                v_unit(kt)
              for dc in range(DCH):
                proj_unit(xq_sb, wq_sb, bq_sb, qT_sb, 0, dc)

              # attention passes; deferred work rides in the kt slots
              if INTERLEAVE:
                for qc in range(NCH):
                    deferred = []
                    if qc + 1 < NCH:
                        deferred += [
                            (lambda dc=dc, q=qc + 1: proj_unit(
                                xq_sb, wq_sb, bq_sb, qT_sb, q, dc))
                            for dc in range(DCH)
                        ]
                    if qc >= 1:
                        deferred += outproj_units(qc - 1)
                    nslots = DCH * KT
                    slots = [[] for _ in range(nslots)]
                    for i, u in enumerate(deferred):
                        slots[(i * nslots) // max(len(deferred), 1)].append(u)
                    for pr in range(DCH):
                        po = new_po(qc, pr)
                        attn_pass(qc, pr, po, slots[pr * KT : (pr + 1) * KT])
                        normalize(qc, pr, po)
                for u in outproj_units(NCH - 1):
                    u()
              else:
                for qc in range(1, NCH):
                    for dc in range(DCH):
                        proj_unit(xq_sb, wq_sb, bq_sb, qT_sb, qc, dc)
                for qc in range(NCH):
                    for pr in range(DCH):
                        po = new_po(qc, pr)
                        attn_pass(qc, pr, po, None)
                        normalize(qc, pr, po)
                for qc in range(NCH):
                    for u in outproj_units(qc):
                        u()

            if DEBUG_DUMP:
                nc.sync.dma_start(out=qTd[:, :, :], in_=qT_sb)
                nc.sync.dma_start(out=kTd[:, :, :], in_=kT_sb)
                nc.sync.dma_start(out=vd[:, :, :, :], in_=v_sb)
                nc.sync.dma_start(out=oTd[:, :, :], in_=oT_sb)

    nc.compile()
    return nc


def _shard_inputs(queries, keys, values, Wq, bq, Wk, bk, Wv, bv):
    """Host-side shard/layout prep: bf16, pre-shaped to SBUF layouts."""
    import ml_dtypes

    bf16 = ml_dtypes.bfloat16
    f32 = np.float32

    def to_pcn(xb):
        # [N, E] -> xT [E, N] -> [p, c, n] with row = c*128 + p
        xT = np.asarray(xb, f32).T.reshape(ECH, P, N).transpose(1, 0, 2)
        return np.ascontiguousarray(xT.astype(bf16))

    xT = {}
    for name, x in (("xqt", queries), ("xkt", keys), ("xvt", values)):
        xT[name] = [to_pcn(x[b]) for b in range(B)]

    def w_pcm(W, rows):
        # W[rows] [DH, E] -> .T [E, DH] -> [p, c, m]
        wt = np.asarray(W, f32)[rows].T.reshape(ECH, P, DH).transpose(1, 0, 2)
        return np.ascontiguousarray(wt.astype(bf16))

    maps = []
    for c in range(NCORES):
        b, hg = c // HG, c % HG
        rows = slice(hg * DH, (hg + 1) * DH)
        m = {
            "xqt": xT["xqt"][b],
            "xkt": xT["xkt"][b],
            "xvt": xT["xvt"][b],
            "wqt": w_pcm(Wq, rows),
            "wkt": w_pcm(Wk, rows),
            "wvt": w_pcm(Wv, rows),
            "bq": np.asarray(bq, f32)[rows].reshape(DCH, P, 1).copy(),
            "bk": np.asarray(bk, f32)[rows].reshape(DCH, P, 1).copy(),
            "bv": np.asarray(bv, f32)[rows].reshape(1, DH).astype(bf16),
        }
        maps.append(m)
    return maps


def _shard_wp(Wp):
    import ml_dtypes

    bf16 = ml_dtypes.bfloat16
    Wp = np.asarray(Wp, np.float32)
    outs = []
    for hg in range(HG):
        rows = slice(hg * DH, (hg + 1) * DH)
        # Wp[:, rows] [E, DH] -> .T [DH, E] -> [p, c, e]
        wt = Wp[:, rows].T.reshape(DCH, P, E).transpose(1, 0, 2)
        outs.append(np.ascontiguousarray(wt.astype(bf16)))
    return outs


def kernel(queries, keys, values, Wq, bq, Wk, bk, Wv, bv, Wp, bp):
    from concourse.bass_utils import run_bass_kernel_spmd

    if "nc" not in _CACHE:
        _CACHE["nc"] = _build_program()
    nc = _CACHE["nc"]

    in_maps = _shard_inputs(queries, keys, values, Wq, bq, Wk, bk, Wv, bv)
    wps = _shard_wp(Wp)
    for c in range(NCORES):
        in_maps[c]["wpt"] = wps[c % HG]

    # First execution can race the host->device input transfers (observed:
    # late param buffers read as zeros mid-kernel).  Run once to warm the
    # device-resident buffers, then take the second run's results.
    run_bass_kernel_spmd(nc, in_maps, list(range(NCORES)))
    res = run_bass_kernel_spmd(nc, in_maps, list(range(NCORES)))

    out = np.zeros((B, N, E), np.float32)
    for c in range(NCORES):
        out[c // HG] += res.results[c]["out"]
    out += np.asarray(bp, np.float32)
    return out


# revision 3
# speedup vs baseline: 1.0932x; 1.0932x over previous
"""Multi-head attention (B=2, N=2048, E=1024, H=16) on 8 Trainium2 NeuronCores.

v2: all-bf16 data path + contiguous host-side layouts.

Sharding: data-parallel over batch (2) x tensor-parallel over head-groups (4
groups of 4 heads).  Core c handles batch c//4 and heads 4*(c%4)..4*(c%4)+3.

Host pre-shapes every tensor into the exact SBUF layout ([partition, ...]
contiguous per partition) and casts to bf16, so each DMA is a handful of
large contiguous descriptors.  The device kernel computes
  qT = Wq_s @ xT + bq_s        (feature-major, bf16 [256, 2048])
  kT = Wk_s @ xT + bk_s
  v  = x @ Wv_s.T + bv_s       (position-major, [2048, 256], bf16)
  eT[kpos, q] per head          (transposed energy, f32 psum, K=64 bf16 mm)
  s = exp(eT)  (bf16)           (no max-subtraction: |logits| < ~60 << 88)
  o  = s.T @ [v | 32]           (32-column yields 32*rowsum in psum row 64)
  oT normalized by 1/(32*rowsum)   (= softmax / sqrt(E) module quirk)
  out_partial = oT.T @ Wp[:, cols].T   (position-major [2048, 1024])
Host sums the 4 head-group partials per batch and adds bp.

All matmuls run in bf16 (fp32 PSUM accumulation); fp32r is ~2x slower on HW.
q-chunk projections and the output projection are emitted interleaved with
the attention inner loop so the PE fills the gaps while ScalarE (exp) paces.
"""

import numpy as np

B, N, E, H = 2, 2048, 1024, 16
D = E // H           # 64
NCORES = 8
HG = 4               # head groups
DH = E // HG         # 256 features per head-group
P = 128
NCH = N // 512       # 4 n-chunks of 512
ECH = E // P         # 8 contraction chunks
DCH = DH // P        # 2 feature chunks per shard
KT = N // P          # 16 key tiles
SCALE_COL = float(E ** 0.5)   # 32.0; row 64 of po = 32*rowsum

INTERLEAVE = True   # ride qproj/outproj units inside the attention kt loop
DEBUG_DUMP = False  # add qT/kT/v/oT dram outputs for HW debugging

_CACHE = {}


def _build_program(repeat=1):
    import concourse.bacc as bacc
    import concourse.tile as tile
    from concourse import mybir

    F32 = mybir.dt.float32
    F32R = mybir.dt.float32r
    BF16 = mybir.dt.bfloat16
    EXP = mybir.ActivationFunctionType.Exp

    nc = bacc.Bacc(None, target_bir_lowering=False, debug=False)

    xqt = nc.declare_dram_parameter("xqt", [P, ECH, N], BF16, isOutput=False)
    xkt = nc.declare_dram_parameter("xkt", [P, ECH, N], BF16, isOutput=False)
    xvt = nc.declare_dram_parameter("xvt", [P, ECH, N], BF16, isOutput=False)
    wqt = nc.declare_dram_parameter("wqt", [P, ECH, DH], BF16, isOutput=False)
    wkt = nc.declare_dram_parameter("wkt", [P, ECH, DH], BF16, isOutput=False)
    wvt = nc.declare_dram_parameter("wvt", [P, ECH, DH], BF16, isOutput=False)
    wpt = nc.declare_dram_parameter("wpt", [P, DCH, E], BF16, isOutput=False)
    bqp = nc.declare_dram_parameter("bq", [DCH, P, 1], F32, isOutput=False)
    bkp = nc.declare_dram_parameter("bk", [DCH, P, 1], F32, isOutput=False)
    bvp = nc.declare_dram_parameter("bv", [1, DH], BF16, isOutput=False)
    # [qc, p, nt, E]: row qc*512 + nt*128 + p lives at out[qc, p, nt, :] so
    # each partition's store is 4*E*4B = 16KB contiguous (one descriptor)
    out = nc.declare_dram_parameter("out", [NCH, P, 4, E], F32, isOutput=True)
    if DEBUG_DUMP:
        qTd = nc.declare_dram_parameter("qTd", [P, DCH, N], BF16, isOutput=True)
        kTd = nc.declare_dram_parameter("kTd", [P, DCH, N], BF16, isOutput=True)
        vd = nc.declare_dram_parameter("vd", [P, KT, HG, D + 1], BF16, isOutput=True)
        oTd = nc.declare_dram_parameter("oTd", [P, DCH, N], BF16, isOutput=True)

    with tile.TileContext(nc) as tc:
        with (
            tc.tile_pool(name="singles", bufs=1) as singles,
            tc.tile_pool(name="spool", bufs=4) as spool,
            tc.tile_pool(name="npool", bufs=2) as npool,
            tc.tile_pool(name="opool", bufs=2) as opool,
            tc.tile_pool(name="pproj", bufs=1, space="PSUM") as pproj,
            tc.tile_pool(name="peps", bufs=2, space="PSUM") as peps,
            tc.tile_pool(name="ppo", bufs=2, space="PSUM") as ppo,
            tc.tile_pool(name="pbc", bufs=1, space="PSUM") as pbc,
        ):
            # ---- persistent tiles ----
            wq_sb = singles.tile([P, ECH, DH], BF16)
            wk_sb = singles.tile([P, ECH, DH], BF16)
            wv_sb = singles.tile([P, ECH, DH], BF16)
            wp_sb = singles.tile([P, DCH, E], BF16)
            bq_sb = singles.tile([P, DCH], F32)
            bk_sb = singles.tile([P, DCH], F32)
            bv_sb = singles.tile([1, DH], BF16)
            ones1 = singles.tile([1, P], BF16)
            nc.vector.memset(ones1, 1.0)
            ones1_f = singles.tile([1, P], F32)
            nc.vector.memset(ones1_f, 1.0)

            xk_sb = singles.tile([P, ECH, N], BF16)
            xv_sb = singles.tile([P, ECH, N], BF16)
            xq_sb = singles.tile([P, ECH, N], BF16)

            qT_sb = singles.tile([P, DCH, N], BF16)
            kT_sb = singles.tile([P, DCH, N], BF16)
            oT_sb = singles.tile([P, DCH, N], BF16)
            v_sb = singles.tile([P, KT, HG, D + 1], BF16)
            nc.vector.memset(v_sb[:, :, :, D : D + 1], SCALE_COL)

            def load_inputs():
                nc.sync.dma_start(out=wk_sb, in_=wkt[:, :, :])
                nc.sync.dma_start(out=wv_sb, in_=wvt[:, :, :])
                for c in range(DCH):
                    nc.sync.dma_start(out=bq_sb[:, c : c + 1], in_=bqp[c])
                    nc.sync.dma_start(out=bk_sb[:, c : c + 1], in_=bkp[c])
                nc.sync.dma_start(out=bv_sb, in_=bvp[:, :])
                nc.sync.dma_start(out=xk_sb, in_=xkt[:, :, :])
                nc.sync.dma_start(out=xv_sb, in_=xvt[:, :, :])
                nc.sync.dma_start(out=wq_sb, in_=wqt[:, :, :])
                nc.sync.dma_start(out=wp_sb, in_=wpt[:, :, :])
                nc.sync.dma_start(out=xq_sb, in_=xqt[:, :, :])

            # ---- emit helpers ----
            def proj_unit(x_sb, w_sb, b_sb, dst, ni, dc):
                """One [128, 512] feature-major projection chunk."""
                ns = slice(ni * 512, (ni + 1) * 512)
                ps = pproj.tile([P, 512], F32, tag="proj", name=f"ps{ni}{dc}")
                for ec in range(ECH):
                    nc.tensor.matmul(
                        ps,
                        w_sb[:, ec, dc * P : (dc + 1) * P],
                        x_sb[:, ec, ns],
                        start=(ec == 0),
                        stop=(ec == ECH - 1),
                    )
                nc.vector.tensor_scalar_add(dst[:, dc, ns], ps, b_sb[:, dc : dc + 1])

            def v_unit(kt):
                """One [128, 256] position-major v tile (with bias)."""
                vps = pproj.tile([P, DH], F32, tag="proj", name=f"vps{kt}")
                nc.tensor.matmul(vps, ones1, bv_sb, start=True, stop=False)
                for ec in range(ECH):
                    nc.tensor.matmul(
                        vps,
                        xv_sb[:, ec, kt * P : (kt + 1) * P],
                        wv_sb[:, ec, :],
                        start=False,
                        stop=(ec == ECH - 1),
                    )
                nc.vector.tensor_copy(
                    v_sb[:, kt, :, 0:D],
                    vps.rearrange("p (h d) -> p h d", h=HG),
                )

            def outproj_units(qc):
                """Deferred output projection for q-chunk qc: 4 n-tile units
                sharing one [P, 4, E] staging tile, stored with one DMA."""
                osb = opool.tile([P, 4, E], F32, tag="osb", name=f"osb{qc}")

                def unit(nt):
                    n0 = qc * 512 + nt * P
                    for ecx in range(2):
                        ops = pproj.tile(
                            [P, 512], F32, tag="proj", name=f"ops{qc}{nt}{ecx}"
                        )
                        for dc in range(DCH):
                            nc.tensor.matmul(
                                ops,
                                oT_sb[:, dc, n0 : n0 + P],
                                wp_sb[:, dc, ecx * 512 : (ecx + 1) * 512],
                                start=(dc == 0),
                                stop=(dc == DCH - 1),
                            )
                        nc.vector.tensor_copy(
                            osb[:, nt, ecx * 512 : (ecx + 1) * 512], ops
                        )
                    if nt == 3:
                        nc.sync.dma_start(out=out[qc], in_=osb)

                return [lambda nt=nt: unit(nt) for nt in range(4)]

            def attn_pass(qc, pr, po, slots=None):
                """Full attention pass over all 16 k-tiles for (qc, pr)."""
                qs = slice(qc * 512, (qc + 1) * 512)
                for kt in range(KT):
                    ks = slice(kt * P, (kt + 1) * P)
                    eps = peps.tile([P, 1024], F32, tag="eps", name=f"e{qc}{pr}{kt}")
                    for hp in range(2):
                        rows = slice(hp * D, (hp + 1) * D)
                        nc.tensor.matmul(
                            eps[:, hp * 512 : (hp + 1) * 512],
                            kT_sb[rows, pr, ks],
                            qT_sb[rows, pr, qs],
                            start=True,
                            stop=True,
                        )
                    sT = spool.tile([P, 1024], BF16, tag="sT", name=f"s{qc}{pr}{kt}")
                    nc.scalar.activation(sT, eps, EXP)
                    for hp in range(2):
                        nc.tensor.matmul(
                            po[hp],
                            v_sb[:, kt, 2 * pr + hp, :],
                            sT[:, hp * 512 : (hp + 1) * 512],
                            start=(kt == 0),
                            stop=(kt == KT - 1),
                        )
                    if slots is not None and kt < len(slots):
                        for u in slots[kt]:
                            u()

            def normalize(qc, pr, po):
                qs = slice(qc * 512, (qc + 1) * 512)
                for hp in range(2):
                    rinv = npool.tile([1, 512], F32, tag="rinv")
                    nc.vector.reciprocal(rinv, po[hp][D : D + 1, :])
                    o_tmp = npool.tile([D, 512], F32, tag="otmp")
                    nc.vector.tensor_copy(o_tmp, po[hp][0:D, :])
                    bc = pbc.tile([D, 512], F32, tag="bc")
                    nc.tensor.matmul(
                        bc, ones1_f[:, 0:D], rinv, start=True, stop=True
                    )
                    nc.vector.tensor_mul(
                        oT_sb[hp * D : (hp + 1) * D, pr, qs], o_tmp, bc
                    )

            def new_po(qc, pr):
                return [
                    ppo.tile([D + 1, 512], F32, tag="po", name=f"po{qc}{pr}{hp}")
                    for hp in range(2)
                ]

            # ---- emission ----
            # k, v, q(chunk 0) projections up front; the Tile scheduler
            # overlaps the x DMAs with the earliest independent matmuls.
            # repeat>1 re-emits the whole body for device-time slope probes.
            def kp(ni, dc):
                return lambda: proj_unit(xk_sb, wk_sb, bk_sb, kT_sb, ni, dc)

            def qp(ni, dc):
                return lambda: proj_unit(xq_sb, wq_sb, bq_sb, qT_sb, ni, dc)

            def vu(kt):
                return lambda: v_unit(kt)

            for _rep in range(repeat):
              load_inputs()
              # minimal prologue: only what (qc0, pr0, kt0..3) needs
              kp(0, 0)(); kp(0, 1)()
              for kt in range(4):
                  v_unit(kt)
              qp(0, 0)(); qp(0, 1)()

              # attention passes; deferred work rides in the kt slots
              if INTERLEAVE:
                # remaining k/v/q projections ride inside the qc0 passes,
                # each emitted 4+ k-tiles before its first consumer
                slots0 = [[] for _ in range(KT)]
                slots0[0] = [kp(1, 0)]
                slots0[1] = [kp(1, 1)]
                slots0[2] = [vu(4), vu(5)]
                slots0[3] = [vu(6), vu(7)]
                slots0[4] = [kp(2, 0)]
                slots0[5] = [kp(2, 1)]
                slots0[6] = [vu(8), vu(9)]
                slots0[7] = [vu(10), vu(11)]
                slots0[8] = [kp(3, 0)]
                slots0[9] = [kp(3, 1)]
                slots0[10] = [vu(12), vu(13)]
                slots0[11] = [vu(14), vu(15)]
                po = new_po(0, 0)
                attn_pass(0, 0, po, slots0)
                normalize(0, 0, po)
                slots1 = [[] for _ in range(KT)]
                slots1[0] = [qp(1, 0)]
                slots1[2] = [qp(1, 1)]
                po = new_po(0, 1)
                attn_pass(0, 1, po, slots1)
                normalize(0, 1, po)

                for qc in range(1, NCH):
                    deferred = []
                    if qc + 1 < NCH:
                        deferred += [qp(qc + 1, dc) for dc in range(DCH)]
                    deferred += outproj_units(qc - 1)
                    nslots = DCH * KT
                    slots = [[] for _ in range(nslots)]
                    for i, u in enumerate(deferred):
                        slots[(i * nslots) // max(len(deferred), 1)].append(u)
                    for pr in range(DCH):
                        po = new_po(qc, pr)
                        attn_pass(qc, pr, po, slots[pr * KT : (pr + 1) * KT])
                        normalize(qc, pr, po)
                for u in outproj_units(NCH - 1):
                    u()
              else:
                for ni in range(1, NCH):
                    for dc in range(DCH):
                        kp(ni, dc)()
                for kt in range(4, KT):
                    v_unit(kt)
                for qc in range(1, NCH):
                    for dc in range(DCH):
                        qp(qc, dc)()
                for qc in range(NCH):
                    for pr in range(DCH):
                        po = new_po(qc, pr)
                        attn_pass(qc, pr, po, None)
                        normalize(qc, pr, po)
                for qc in range(NCH):
                    for u in outproj_units(qc):
                        u()

            if DEBUG_DUMP:
                nc.sync.dma_start(out=qTd[:, :, :], in_=qT_sb)
                nc.sync.dma_start(out=kTd[:, :, :], in_=kT_sb)
                nc.sync.dma_start(out=vd[:, :, :, :], in_=v_sb)
                nc.sync.dma_start(out=oTd[:, :, :], in_=oT_sb)

    nc.compile()
    return nc


def _shard_inputs(queries, keys, values, Wq, bq, Wk, bk, Wv, bv):
    """Host-side shard/layout prep: bf16, pre-shaped to SBUF layouts."""
    import ml_dtypes

    bf16 = ml_dtypes.bfloat16
    f32 = np.float32

    def to_pcn(xb):
        # [N, E] -> xT [E, N] -> [p, c, n] with row = c*128 + p
        xT = np.asarray(xb, f32).T.reshape(ECH, P, N).transpose(1, 0, 2)
        return np.ascontiguousarray(xT.astype(bf16))

    xT = {}
    for name, x in (("xqt", queries), ("xkt", keys), ("xvt", values)):
        xT[name] = [to_pcn(x[b]) for b in range(B)]

    def w_pcm(W, rows):
        # W[rows] [DH, E] -> .T [E, DH] -> [p, c, m]
        wt = np.asarray(W, f32)[rows].T.reshape(ECH, P, DH).transpose(1, 0, 2)
        return np.ascontiguousarray(wt.astype(bf16))

    maps = []
    for c in range(NCORES):
        b, hg = c // HG, c % HG
        rows = slice(hg * DH, (hg + 1) * DH)
        m = {
            "xqt": xT["xqt"][b],
            "xkt": xT["xkt"][b],
            "xvt": xT["xvt"][b],
            "wqt": w_pcm(Wq, rows),
            "wkt": w_pcm(Wk, rows),
            "wvt": w_pcm(Wv, rows),
            "bq": np.asarray(bq, f32)[rows].reshape(DCH, P, 1).copy(),
            "bk": np.asarray(bk, f32)[rows].reshape(DCH, P, 1).copy(),
            "bv": np.asarray(bv, f32)[rows].reshape(1, DH).astype(bf16),
        }
        maps.append(m)
    return maps


def _shard_wp(Wp):
    import ml_dtypes

    bf16 = ml_dtypes.bfloat16
    Wp = np.asarray(Wp, np.float32)
    outs = []
    for hg in range(HG):
        rows = slice(hg * DH, (hg + 1) * DH)
        # Wp[:, rows] [E, DH] -> .T [DH, E] -> [p, c, e]
        wt = Wp[:, rows].T.reshape(DCH, P, E).transpose(1, 0, 2)
        outs.append(np.ascontiguousarray(wt.astype(bf16)))
    return outs


def _unshard_out(arr):
    """[qc, p, nt, E] device layout -> [N, E]."""
    return arr.transpose(0, 2, 1, 3).reshape(N, E)


def kernel(queries, keys, values, Wq, bq, Wk, bk, Wv, bv, Wp, bp):
    from concourse.bass_utils import run_bass_kernel_spmd

    if "nc" not in _CACHE:
        _CACHE["nc"] = _build_program()
    nc = _CACHE["nc"]

    in_maps = _shard_inputs(queries, keys, values, Wq, bq, Wk, bk, Wv, bv)
    wps = _shard_wp(Wp)
    for c in range(NCORES):
        in_maps[c]["wpt"] = wps[c % HG]

    # First execution can race the host->device input transfers (observed:
    # late param buffers read as zeros mid-kernel).  Run once to warm the
    # device-resident buffers, then take the second run's results.
    run_bass_kernel_spmd(nc, in_maps, list(range(NCORES)))
    res = run_bass_kernel_spmd(nc, in_maps, list(range(NCORES)))

    out = np.zeros((B, N, E), np.float32)
    for c in range(NCORES):
        out[c // HG] += _unshard_out(res.results[c]["out"])
    out += np.asarray(bp, np.float32)
    return out
